# revision 30
# baseline (speedup 1.0000x reference)
"""Causal single-head attention (N=4096, D=F=1024) on 8 TRN2 NeuronCores.

Sequence-parallel with causal load balancing: query tiles (128 rows) are
assigned round-robin — core c owns tiles {c, 8+c, 16+c, 24+c}, one per
"slot" k=0..3.  Slot k only attends key tiles [0, 8*(k+1)), so attention
matmul work drops to 80/128 of the dense-causal-ignoring version while
the SPMD program stays uniform across cores (the per-core diagonal
position is handled by per-core mask DATA, not control flow).

Two SPMD launches:
  A) QKV projection — each core computes q/k/v for its own 4 query tiles
     (weights replicated; host pre-transposes to contraction-major).
  B) attention + output projection — chunk-major: for key tile m the
     scores matmul covers all still-eligible slots at once, so the free
     dim is 512/384/256/128 (wide free keeps the PE's LDWEIGHTS hidden).
     k/v live SBUF-resident and are shared across slots (nested key
     ranges).  att@v runs as two ft-half passes to fit zT in 4 PSUM
     banks.  Row sums come from a ones-column matmul; the reciprocal is
     transposed to query-partition form via a DRAM round trip hidden
     under the output-projection matmuls.

Matmul operands are bf16 (f32 PSUM accumulation); all big DMAs are
host-pre-blocked so each is ~128 descriptors of >=2KB contiguous.
"""

import sys

try:
    import concourse.bass as bass
except ImportError:  # pragma: no cover
    sys.path.insert(0, "/opt/trn_rl_repo")
    import concourse.bass as bass

import ml_dtypes
import numpy as np

import concourse.mybir as mybir
import concourse.tile as tile
from concourse import bacc
from concourse.bass_utils import run_bass_kernel_spmd

N, D, F = 4096, 1024, 1024
C = 8              # cores
NL = N // C        # 512 query rows per core
P = 128
SCALE = 1.0 / float(np.sqrt(np.float32(F)))

F32 = mybir.dt.float32
MM_DT = mybir.dt.bfloat16  # matmul operand dtype (PSUM accumulation stays f32)

DT = D // P        # 8 contraction tiles
FT = F // P        # 8 f tiles
MT = N // P        # 32 key tiles
SLOTS = NL // P    # 4 query tiles (slots) per core
CI = MT // 8       # 4 key chunks of 8 key tiles

# Filled with [launchA_ns, launchB_ns] when BASS_TRACE=1 profiling is active.
LAST_EXEC_NS = [None, None]
LAST_RESULTS = [None, None]

_CACHE = {}


def _build_qkv():
    nc = bacc.Bacc(None, target_bir_lowering=False)
    xT = nc.dram_tensor("xT", [P, DT, NL], MM_DT, kind="ExternalInput")
    wqb = nc.dram_tensor("wqb", [FT, P, DT, P], MM_DT, kind="ExternalInput")
    wkb = nc.dram_tensor("wkb", [FT, P, DT, P], MM_DT, kind="ExternalInput")
    wvb = nc.dram_tensor("wvb", [2, P, DT, 512], MM_DT, kind="ExternalInput")
    bq = nc.dram_tensor("bq", [P, FT], F32, kind="ExternalInput")
    bk = nc.dram_tensor("bk", [P, FT], F32, kind="ExternalInput")
    bvB = nc.dram_tensor("bvB", [P, F], F32, kind="ExternalInput")
    qT_o = nc.dram_tensor("qT_o", [F, NL], MM_DT, kind="ExternalOutput")
    kT_o = nc.dram_tensor("kT_o", [F, NL], MM_DT, kind="ExternalOutput")
    v_o = nc.dram_tensor("v_o", [NL, F], MM_DT, kind="ExternalOutput")

    with tile.TileContext(nc) as tc:
        with (
            tc.tile_pool(name="singles", bufs=1) as singles,
            tc.tile_pool(name="weights", bufs=8) as weights,
            tc.tile_pool(name="osb", bufs=6) as opool,
            tc.tile_pool(name="psum", bufs=6, space="PSUM") as psum,
        ):
            warm = singles.tile([P, NL], MM_DT)
            nc.vector.memset(warm, 0.0)
            wps = psum.tile([P, NL], F32, tag="ps")
            for wi in range(24):
                nc.tensor.matmul(
                    wps,
                    warm[:, :P],
                    warm,
                    start=(wi == 0),
                    stop=(wi == 23),
                )
            # xT split 4-way across both fast queues so it lands early
            xT_sb = singles.tile([P, DT, NL], MM_DT)
            nc.sync.dma_start(out=xT_sb[:, :2, :], in_=xT.ap()[:, :2, :])
            nc.scalar.dma_start(out=xT_sb[:, 2:4, :], in_=xT.ap()[:, 2:4, :])
            nc.sync.dma_start(out=xT_sb[:, 4:6, :], in_=xT.ap()[:, 4:6, :])
            nc.scalar.dma_start(out=xT_sb[:, 6:, :], in_=xT.ap()[:, 6:, :])
            bq_sb = singles.tile([P, FT], F32)
            nc.scalar.dma_start(out=bq_sb, in_=bq.ap())
            bk_sb = singles.tile([P, FT], F32)
            nc.scalar.dma_start(out=bk_sb, in_=bk.ap())
            bvB_sb = singles.tile([P, F], F32)
            nc.scalar.dma_start(out=bvB_sb, in_=bvB.ap())

            # q.T / k.T : out[f_tile, n] = sum_d wT[d, f] * xT[d, n]
            # weights streamed per-f-tile; wq on sync, wk on scalar so the
            # two input streams share the bus evenly
            for w_t, b_sb, out_t, w_eng, o_eng in (
                (wqb, bq_sb, qT_o, nc.sync, nc.gpsimd),
                (wkb, bk_sb, kT_o, nc.scalar, nc.gpsimd),
            ):
                for ft in range(FT):
                    wc = weights.tile([P, DT, P], MM_DT, tag="wc")
                    w_eng.dma_start(out=wc, in_=w_t.ap()[ft])
                    ps = psum.tile([P, NL], F32, tag="ps")
                    for dt_i in range(DT):
                        nc.tensor.matmul(
                            ps,
                            wc[:, dt_i, :],
                            xT_sb[:, dt_i, :],
                            start=(dt_i == 0),
                            stop=(dt_i == DT - 1),
                        )
                    osb = opool.tile([P, NL], MM_DT, tag="osb")
                    nc.vector.tensor_scalar_add(
                        out=osb, in0=ps, scalar1=b_sb[:, ft : ft + 1]
                    )
                    o_eng.dma_start(
                        out=out_t.ap()[ft * P : (ft + 1) * P, :], in_=osb
                    )

            # v : out[m_tile, f] = sum_d xT[d, m] * wvT[d, f]
            for fc in range(2):
                fs = slice(fc * 512, (fc + 1) * 512)
                wvc = weights.tile([P, DT, 512], MM_DT, tag="wvc")
                nc.sync.dma_start(out=wvc, in_=wvb.ap()[fc])
                for mi in range(SLOTS):
                    ps = psum.tile([P, 512], F32, tag="ps")
                    for dt_i in range(DT):
                        nc.tensor.matmul(
                            ps,
                            xT_sb[:, dt_i, mi * P : (mi + 1) * P],
                            wvc[:, dt_i, :],
                            start=(dt_i == 0),
                            stop=(dt_i == DT - 1),
                        )
                    vsb = opool.tile([P, 512], MM_DT, tag="osb")
                    nc.vector.tensor_add(out=vsb, in0=ps, in1=bvB_sb[:, fs])
                    nc.scalar.dma_start(
                        out=v_o.ap()[mi * P : (mi + 1) * P, fs], in_=vsb
                    )
    nc.finalize()
    return nc


def _build_attn():
    nc = bacc.Bacc(None, target_bir_lowering=False)
    qT = nc.dram_tensor("qT", [P, FT, NL], MM_DT, kind="ExternalInput")
    # kb[ci, p, u, ft, j] = k[(8*ci+u)*128 + j, ft*128 + p]
    kb = nc.dram_tensor("kb", [CI, P, 8, FT, P], MM_DT, kind="ExternalInput")
    # vb[ci, p, u, f] = v[(8*ci+u)*128 + p, f]
    vb = nc.dram_tensor("vb", [CI, P, 8, F], MM_DT, kind="ExternalInput")
    # maskb[p, u, q]: per-core diagonal-region masks (ones / tril / zeros)
    maskb = nc.dram_tensor("maskb", [P, 8, P], MM_DT, kind="ExternalInput")
    # projTb[p, t, f] = proj_w.T[t*128+p, f]
    projTb = nc.dram_tensor("projTb", [P, FT, F], MM_DT, kind="ExternalInput")
    pbB = nc.dram_tensor("pbB", [P, F], F32, kind="ExternalInput")
    out_o = nc.dram_tensor("out_o", [NL, F], MM_DT, kind="ExternalOutput")

    with tile.TileContext(nc) as tc:
        with (
            tc.tile_pool(name="singles", bufs=1) as singles,
            tc.tile_pool(name="osb", bufs=3) as opool,
            tc.tile_pool(name="sps", bufs=3, space="PSUM") as spsum,
            tc.tile_pool(name="zps", bufs=4, space="PSUM") as zpsum,
            tc.tile_pool(name="rps", bufs=1, space="PSUM") as rpsum,
            tc.tile_pool(name="dram", bufs=1, space="DRAM") as drampool,
        ):
            warm = singles.tile([P, 512], MM_DT)
            nc.vector.memset(warm, 0.0)
            wps = zpsum.tile([P, 512], F32, tag="zps")
            NWARM = 14  # sized to cover the first-window DMA latency
            for wi in range(NWARM):
                nc.tensor.matmul(
                    wps,
                    warm[:, :P],
                    warm,
                    start=(wi == 0),
                    stop=(wi == NWARM - 1),
                )

            # ---- resident inputs.  The critical first-window tensors (masks,
            # q, k chunk 0, v chunk 0) are sub-chunked so the first score
            # matmuls start ~4 us in and stream behind the DMAs.
            # q rides first on the sync queue — it gates the very first
            # scores matmul and sync's DGE has the lowest start latency
            qT_sb = singles.tile([P, FT, NL], MM_DT)
            nc.sync.dma_start(out=qT_sb, in_=qT.ap())
            masks_sb = singles.tile([P, 8, P], MM_DT)
            nc.scalar.dma_start(out=masks_sb, in_=maskb.ap())
            # 1 MB sub-chunks interleaved across the two big DMA queues in
            # exact consumption order, so the chunk needed next always has
            # the full bus: sync moves k0,v1,k2,v3 while gpsimd moves
            # v0,k1,v2,k3.
            k_sb = [
                singles.tile([P, 8, FT, P], MM_DT, name=f"k_sb{ci}")
                for ci in range(CI)
            ]
            v_sb = [
                singles.tile([P, 8, F], MM_DT, name=f"v_sb{ci}")
                for ci in range(CI)
            ]
            for ci in range(CI):
                k_eng = nc.sync if ci % 2 == 0 else nc.gpsimd
                v_eng = nc.gpsimd if ci % 2 == 0 else nc.sync
                if ci == 0:
                    # head-split: the first key/value tiles gate the first
                    # real matmuls right after warmup
                    for lo, hi in ((0, 1), (1, 2), (2, 4), (4, 8)):
                        k_eng.dma_start(
                            out=k_sb[ci][:, lo:hi], in_=kb.ap()[ci, :, lo:hi]
                        )
                        v_eng.dma_start(
                            out=v_sb[ci][:, lo:hi], in_=vb.ap()[ci, :, lo:hi]
                        )
                else:
                    k_eng.dma_start(out=k_sb[ci][:, :4], in_=kb.ap()[ci, :, :4])
                    k_eng.dma_start(out=k_sb[ci][:, 4:], in_=kb.ap()[ci, :, 4:])
                    v_eng.dma_start(out=v_sb[ci][:, :4], in_=vb.ap()[ci, :, :4])
                    v_eng.dma_start(out=v_sb[ci][:, 4:], in_=vb.ap()[ci, :, 4:])
            # gpsimd's DMA queue is serial, so these naturally wait behind
            # the v transfers — out of the critical first-window bandwidth
            projT_sb = singles.tile([P, FT, F], MM_DT)
            nc.gpsimd.dma_start(out=projT_sb, in_=projTb.ap())
            pbB_sb = singles.tile([P, F], F32)
            nc.gpsimd.dma_start(out=pbB_sb, in_=pbB.ap())
            ones_sb = singles.tile([P, 1], MM_DT)
            nc.vector.memset(ones_sb, 1.0)

            # pt arenas (bf16 attention weights), one per key chunk
            pt_ar = [
                singles.tile([P, 8, (CI - ci) * P], MM_DT, name=f"pt{ci}")
                for ci in range(CI)
            ]
            # zT arena: z^T[f, q] bf16, [P, ft, 512]
            zT_sb = singles.tile([P, FT, NL], MM_DT)
            rps = rpsum.tile([1, NL], F32)
            zA = [zpsum.tile([P, NL], F32, tag="zps", name=f"zA{h}") for h in range(4)]

            def scores(m):
                ci, u = divmod(m, 8)
                W = (CI - ci) * P
                ps = spsum.tile([P, W], F32, tag="sps")
                for ft in range(FT):
                    nc.tensor.matmul(
                        ps,
                        k_sb[ci][:, u, ft, :],
                        qT_sb[:, ft, ci * P : NL],
                        start=(ft == 0),
                        stop=(ft == FT - 1),
                    )
                pt = pt_ar[ci][:, u, :]
                nc.scalar.activation(
                    out=pt,
                    in_=ps,
                    func=mybir.ActivationFunctionType.Exp,
                    scale=SCALE,
                )
                # mask only the first 128 columns (slot ci — its diagonal chunk)
                nc.vector.tensor_mul(
                    out=pt_ar[ci][:, u, :P],
                    in0=pt_ar[ci][:, u, :P],
                    in1=masks_sb[:, u, :],
                )

            def attv(m, zt, fts):
                ci, u = divmod(m, 8)
                pt = pt_ar[ci][:, u, :]
                for i, ft in enumerate(fts):
                    nc.tensor.matmul(
                        zt[i][:, ci * P : NL],
                        v_sb[ci][:, u, ft * P : (ft + 1) * P],
                        pt,
                        start=(m == 0),
                        stop=(m == MT - 1),
                        skip_group_check=True,
                    )

            def rowsum(m):
                ci, u = divmod(m, 8)
                nc.tensor.matmul(
                    rps[:, ci * P : NL],
                    ones_sb,
                    pt_ar[ci][:, u, :],
                    start=(m == 0),
                    stop=(m == MT - 1),
                    skip_group_check=True,
                )

            # ---- pass A: scores + exp + mask + att@v (ft 0..3) + rowsums
            scores(0)
            for m in range(1, MT):
                scores(m)
                attv(m - 1, zA, range(4))
                rowsum(m - 1)
            attv(MT - 1, zA, range(4))
            rowsum(MT - 1)

            # row sums -> [q-part, slot] via DRAM round trip, then reciprocal
            rsum_row = singles.tile([1, NL], F32)
            nc.vector.tensor_copy(out=rsum_row, in_=rps)
            scratch = drampool.tile([1, NL], F32)
            nc.scalar.dma_start(out=scratch, in_=rsum_row)
            rsum_np = singles.tile([P, SLOTS], F32)
            nc.scalar.dma_start(
                out=rsum_np, in_=scratch[0].rearrange("(t p) -> p t", p=P)
            )
            recip_np = singles.tile([P, SLOTS], F32)
            nc.vector.reciprocal(out=recip_np, in_=rsum_np)

            # drain zT (ft 0..3)
            for h in range(4):
                if h % 2 == 0:
                    nc.vector.tensor_copy(out=zT_sb[:, h, :], in_=zA[h])
                else:
                    nc.scalar.activation(
                        out=zT_sb[:, h, :],
                        in_=zA[h],
                        func=mybir.ActivationFunctionType.Copy,
                    )

            # ---- pass B: att@v (ft 4..7)
            zB = [zpsum.tile([P, NL], F32, tag="zps", name=f"zB{h}") for h in range(4)]
            for m in range(MT):
                attv(m, zB, range(4, FT))
            for h in range(4):
                if h % 2 == 0:
                    nc.vector.tensor_copy(out=zT_sb[:, 4 + h, :], in_=zB[h])
                else:
                    nc.scalar.activation(
                        out=zT_sb[:, 4 + h, :],
                        in_=zB[h],
                        func=mybir.ActivationFunctionType.Copy,
                    )

            # ---- out[n, o] = (zT/rowsum) @ projT + pb
            for kk in range(SLOTS):
                for oc in range(2):
                    os_ = slice(oc * 512, (oc + 1) * 512)
                    ops = zpsum.tile([P, 512], F32, tag="zps", name="ops")
                    for ft in range(FT):
                        nc.tensor.matmul(
                            ops,
                            zT_sb[:, ft, kk * P : (kk + 1) * P],
                            projT_sb[:, ft, os_],
                            start=(ft == 0),
                            stop=(ft == FT - 1),
                        )
                    osb = opool.tile([P, 512], MM_DT, tag="osb")
                    nc.vector.scalar_tensor_tensor(
                        out=osb,
                        in0=ops,
                        scalar=recip_np[:, kk : kk + 1],
                        in1=pbB_sb[:, os_],
                        op0=mybir.AluOpType.mult,
                        op1=mybir.AluOpType.add,
                    )
                    nc.sync.dma_start(
                        out=out_o.ap()[kk * P : (kk + 1) * P, os_], in_=osb
                    )
    nc.finalize()
    return nc


def _get_programs():
    if "qkv" not in _CACHE:
        _CACHE["qkv"] = _build_qkv()
        _CACHE["attn"] = _build_attn()
    return _CACHE["qkv"], _CACHE["attn"]


def _c(a):
    return np.ascontiguousarray(a, dtype=np.float32)


def _b(a):
    return np.ascontiguousarray(np.asarray(a, dtype=np.float32).astype(ml_dtypes.bfloat16))


def kernel(x, wq_w, wq_b, wk_w, wk_b, wv_w, wv_b, proj_w, proj_b):
    x = np.asarray(x, dtype=np.float32)
    nc_qkv, nc_attn = _get_programs()

    # ---- launch A: QKV projection; core c owns query tiles {c, 8+c, 16+c, 24+c}
    # blocked weight layouts: [FT, P, DT, P] so every chunk DMA is linear
    wqb = _b(np.asarray(wq_w).T.reshape(DT, P, FT, P).transpose(2, 1, 0, 3))
    wkb = _b(np.asarray(wk_w).T.reshape(DT, P, FT, P).transpose(2, 1, 0, 3))
    wvb = _b(np.asarray(wv_w).T.reshape(DT, P, 2, 512).transpose(2, 1, 0, 3))
    bq_pb = _c(np.asarray(wq_b).reshape(FT, P).T)   # [P, FT]
    bk_pb = _c(np.asarray(wk_b).reshape(FT, P).T)
    bvB = _c(np.broadcast_to(np.asarray(wv_b), (P, F)))
    x_t = x.reshape(MT, P, D)                       # [tile, row, d]
    in_a = []
    for c in range(C):
        rows = x_t[c::C].reshape(NL, D)             # tiles c, 8+c, 16+c, 24+c
        xT_blk = _b(rows.T.reshape(DT, P, NL).transpose(1, 0, 2))
        in_a.append(
            {
                "xT": xT_blk,
                "wqb": wqb,
                "wkb": wkb,
                "wvb": wvb,
                "bq": bq_pb,
                "bk": bk_pb,
                "bvB": bvB,
            }
        )
    res_a = run_bass_kernel_spmd(nc_qkv, in_a, core_ids=list(range(C)))
    LAST_EXEC_NS[0] = res_a.exec_time_ns
    LAST_RESULTS[0] = res_a

    # reassemble full k/v in natural row order (tile index = 8*s + c)
    kT_all = np.stack(
        [np.asarray(res_a.results[c]["kT_o"]).reshape(F, SLOTS, P) for c in range(C)]
    )  # [c, F, s, P]
    kT_full = kT_all.transpose(1, 2, 0, 3).reshape(F, N)
    v_all = np.stack(
        [np.asarray(res_a.results[c]["v_o"]).reshape(SLOTS, P, F) for c in range(C)]
    )  # [c, s, P, F]
    v_full = v_all.transpose(1, 0, 2, 3).reshape(N, F)

    # ---- launch B: attention + projection (chunk-blocked, replicated k/v)
    # kb[ci, p, u, ft, j] = kT_full[ft*128 + p, (8*ci+u)*128 + j]
    kb = np.ascontiguousarray(
        kT_full.reshape(FT, P, CI, 8, P).transpose(2, 1, 3, 0, 4)
    )
    vb = np.ascontiguousarray(v_full.reshape(CI, 8, P, F).transpose(0, 2, 1, 3))
    projTb = _b(np.asarray(proj_w).T.reshape(FT, P, F).transpose(1, 0, 2))
    pbB = _c(np.broadcast_to(np.asarray(proj_b), (P, F)))
    tri = (np.arange(P)[:, None] <= np.arange(P)[None, :])  # key p <= query q
    in_b = []
    for c in range(C):
        qT_blk = np.ascontiguousarray(
            np.asarray(res_a.results[c]["qT_o"]).reshape(FT, P, NL).transpose(1, 0, 2)
        )
        maskb = np.zeros((P, 8, P), dtype=ml_dtypes.bfloat16)
        maskb[:, :c, :] = 1.0
        maskb[:, c, :] = tri.astype(ml_dtypes.bfloat16)
        in_b.append(
            {
                "qT": qT_blk,
                "kb": kb,
                "vb": vb,
                "maskb": maskb,
                "projTb": projTb,
                "pbB": pbB,
            }
        )
    res_b = run_bass_kernel_spmd(nc_attn, in_b, core_ids=list(range(C)))
    LAST_EXEC_NS[1] = res_b.exec_time_ns
    LAST_RESULTS[1] = res_b

    out_all = np.stack(
        [
            np.asarray(res_b.results[c]["out_o"]).astype(np.float32).reshape(SLOTS, P, F)
            for c in range(C)
        ]
    )  # [c, k, P, F]
    return out_all.transpose(1, 0, 2, 3).reshape(N, F)


# revision 31
# speedup vs baseline: 1.1214x; 1.1214x over previous
"""Causal single-head attention (N=4096, D=F=1024) on 8 TRN2 NeuronCores.

Sequence-parallel with causal load balancing: query tiles (128 rows) are
assigned round-robin — core c owns tiles {c, 8+c, 16+c, 24+c}, one per
"slot" k=0..3.  Slot k only attends key tiles [0, 8*(k+1)), so attention
matmul work drops to 80/128 of the dense-causal-ignoring version while
the SPMD program stays uniform across cores (the per-core diagonal
position is handled by per-core mask DATA, not control flow).

Two SPMD launches:
  A) QKV projection — each core computes q/k/v for its own 4 query tiles
     (weights replicated; host pre-transposes to contraction-major).
  B) attention + output projection — chunk-major: for key tile m the
     scores matmul covers all still-eligible slots at once, so the free
     dim is 512/384/256/128 (wide free keeps the PE's LDWEIGHTS hidden).
     k/v live SBUF-resident and are shared across slots (nested key
     ranges).  att@v runs as two ft-half passes to fit zT in 4 PSUM
     banks.  Row sums come from a ones-column matmul; the reciprocal is
     transposed to query-partition form via a DRAM round trip hidden
     under the output-projection matmuls.

Matmul operands are bf16 (f32 PSUM accumulation); all big DMAs are
host-pre-blocked so each is ~128 descriptors of >=2KB contiguous.
"""

import sys

try:
    import concourse.bass as bass
except ImportError:  # pragma: no cover
    sys.path.insert(0, "/opt/trn_rl_repo")
    import concourse.bass as bass

import ml_dtypes
import numpy as np

import concourse.mybir as mybir
import concourse.tile as tile
from concourse import bacc
from concourse.bass_utils import run_bass_kernel_spmd

N, D, F = 4096, 1024, 1024
C = 8              # cores
NL = N // C        # 512 query rows per core
P = 128
SCALE = 1.0 / float(np.sqrt(np.float32(F)))

F32 = mybir.dt.float32
MM_DT = mybir.dt.bfloat16  # matmul operand dtype (PSUM accumulation stays f32)

DT = D // P        # 8 contraction tiles
FT = F // P        # 8 f tiles
MT = N // P        # 32 key tiles
SLOTS = NL // P    # 4 query tiles (slots) per core
CI = MT // 8       # 4 key chunks of 8 key tiles

# Filled with [launchA_ns, launchB_ns] when BASS_TRACE=1 profiling is active.
LAST_EXEC_NS = [None, None]
LAST_RESULTS = [None, None]

_CACHE = {}


def _build_qkv():
    nc = bacc.Bacc(None, target_bir_lowering=False)
    xT = nc.dram_tensor("xT", [P, DT, NL], MM_DT, kind="ExternalInput")
    wqb = nc.dram_tensor("wqb", [FT, P, DT, P], MM_DT, kind="ExternalInput")
    wkb = nc.dram_tensor("wkb", [FT, P, DT, P], MM_DT, kind="ExternalInput")
    wvb = nc.dram_tensor("wvb", [2, P, DT, 512], MM_DT, kind="ExternalInput")
    bq = nc.dram_tensor("bq", [P, FT], F32, kind="ExternalInput")
    bk = nc.dram_tensor("bk", [P, FT], F32, kind="ExternalInput")
    bvB = nc.dram_tensor("bvB", [P, F], F32, kind="ExternalInput")
    qT_o = nc.dram_tensor("qT_o", [F, NL], MM_DT, kind="ExternalOutput")
    kT_o = nc.dram_tensor("kT_o", [F, NL], MM_DT, kind="ExternalOutput")
    v_o = nc.dram_tensor("v_o", [NL, F], MM_DT, kind="ExternalOutput")

    with tile.TileContext(nc) as tc:
        with (
            tc.tile_pool(name="singles", bufs=1) as singles,
            tc.tile_pool(name="weights", bufs=8) as weights,
            tc.tile_pool(name="osb", bufs=6) as opool,
            tc.tile_pool(name="psum", bufs=6, space="PSUM") as psum,
        ):
            warm = singles.tile([P, NL], MM_DT)
            nc.vector.memset(warm, 0.0)
            wps = psum.tile([P, NL], F32, tag="ps")
            for wi in range(24):
                nc.tensor.matmul(
                    wps,
                    warm[:, :P],
                    warm,
                    start=(wi == 0),
                    stop=(wi == 23),
                )
            # xT split 4-way across both fast queues so it lands early
            xT_sb = singles.tile([P, DT, NL], MM_DT)
            nc.sync.dma_start(out=xT_sb[:, :2, :], in_=xT.ap()[:, :2, :])
            nc.scalar.dma_start(out=xT_sb[:, 2:4, :], in_=xT.ap()[:, 2:4, :])
            nc.sync.dma_start(out=xT_sb[:, 4:6, :], in_=xT.ap()[:, 4:6, :])
            nc.scalar.dma_start(out=xT_sb[:, 6:, :], in_=xT.ap()[:, 6:, :])
            bq_sb = singles.tile([P, FT], F32)
            nc.scalar.dma_start(out=bq_sb, in_=bq.ap())
            bk_sb = singles.tile([P, FT], F32)
            nc.scalar.dma_start(out=bk_sb, in_=bk.ap())
            bvB_sb = singles.tile([P, F], F32)
            nc.scalar.dma_start(out=bvB_sb, in_=bvB.ap())

            # q.T / k.T : out[f_tile, n] = sum_d wT[d, f] * xT[d, n]
            # weights streamed per-f-tile; wq on sync, wk on scalar so the
            # two input streams share the bus evenly
            for w_t, b_sb, out_t, w_eng, o_eng in (
                (wqb, bq_sb, qT_o, nc.sync, nc.gpsimd),
                (wkb, bk_sb, kT_o, nc.scalar, nc.gpsimd),
            ):
                for ft in range(FT):
                    wc = weights.tile([P, DT, P], MM_DT, tag="wc")
                    w_eng.dma_start(out=wc, in_=w_t.ap()[ft])
                    ps = psum.tile([P, NL], F32, tag="ps")
                    for dt_i in range(DT):
                        nc.tensor.matmul(
                            ps,
                            wc[:, dt_i, :],
                            xT_sb[:, dt_i, :],
                            start=(dt_i == 0),
                            stop=(dt_i == DT - 1),
                        )
                    osb = opool.tile([P, NL], MM_DT, tag="osb")
                    nc.vector.tensor_scalar_add(
                        out=osb, in0=ps, scalar1=b_sb[:, ft : ft + 1]
                    )
                    o_eng.dma_start(
                        out=out_t.ap()[ft * P : (ft + 1) * P, :], in_=osb
                    )

            # v : out[m_tile, f] = sum_d xT[d, m] * wvT[d, f]
            for fc in range(2):
                fs = slice(fc * 512, (fc + 1) * 512)
                wvc = weights.tile([P, DT, 512], MM_DT, tag="wvc")
                nc.sync.dma_start(out=wvc, in_=wvb.ap()[fc])
                for mi in range(SLOTS):
                    ps = psum.tile([P, 512], F32, tag="ps")
                    for dt_i in range(DT):
                        nc.tensor.matmul(
                            ps,
                            xT_sb[:, dt_i, mi * P : (mi + 1) * P],
                            wvc[:, dt_i, :],
                            start=(dt_i == 0),
                            stop=(dt_i == DT - 1),
                        )
                    vsb = opool.tile([P, 512], MM_DT, tag="osb")
                    nc.vector.tensor_add(out=vsb, in0=ps, in1=bvB_sb[:, fs])
                    nc.scalar.dma_start(
                        out=v_o.ap()[mi * P : (mi + 1) * P, fs], in_=vsb
                    )
    nc.finalize()
    return nc


def _build_attn():
    nc = bacc.Bacc(None, target_bir_lowering=False)
    qT = nc.dram_tensor("qT", [P, FT, NL], MM_DT, kind="ExternalInput")
    # kb[ci, p, u, ft, j] = k[(8*ci+u)*128 + j, ft*128 + p]
    kb = nc.dram_tensor("kb", [CI, P, 8, FT, P], MM_DT, kind="ExternalInput")
    # vb[ci, p, u, f] = v[(8*ci+u)*128 + p, f]
    vb = nc.dram_tensor("vb", [CI, P, 8, F], MM_DT, kind="ExternalInput")
    # maskb[p, u, q]: per-core diagonal-region masks (ones / tril / zeros)
    maskb = nc.dram_tensor("maskb", [P, 8, P], MM_DT, kind="ExternalInput")
    # projTb[p, t, f] = proj_w.T[t*128+p, f]
    projTb = nc.dram_tensor("projTb", [P, FT, F], MM_DT, kind="ExternalInput")
    pbB = nc.dram_tensor("pbB", [P, F], F32, kind="ExternalInput")
    out_o = nc.dram_tensor("out_o", [NL, F], MM_DT, kind="ExternalOutput")

    with tile.TileContext(nc) as tc:
        with (
            tc.tile_pool(name="singles", bufs=1) as singles,
            tc.tile_pool(name="osb", bufs=3) as opool,
            tc.tile_pool(name="sps", bufs=3, space="PSUM") as spsum,
            tc.tile_pool(name="zps", bufs=4, space="PSUM") as zpsum,
            tc.tile_pool(name="rps", bufs=1, space="PSUM") as rpsum,
            tc.tile_pool(name="dram", bufs=1, space="DRAM") as drampool,
        ):
            warm = singles.tile([P, 512], MM_DT)
            nc.vector.memset(warm, 0.0)
            wps = zpsum.tile([P, 512], F32, tag="zps")
            NWARM = 20  # sized to cover the first-window DMA latency
            for wi in range(NWARM):
                nc.tensor.matmul(
                    wps,
                    warm[:, :P],
                    warm,
                    start=(wi == 0),
                    stop=(wi == NWARM - 1),
                )

            # ---- resident inputs.  The critical first-window tensors (masks,
            # q, k chunk 0, v chunk 0) are sub-chunked so the first score
            # matmuls start ~4 us in and stream behind the DMAs.
            # q rides first on the sync queue — it gates the very first
            # scores matmul and sync's DGE has the lowest start latency
            qT_sb = singles.tile([P, FT, NL], MM_DT)
            nc.sync.dma_start(out=qT_sb, in_=qT.ap())
            masks_sb = singles.tile([P, 8, P], MM_DT)
            nc.scalar.dma_start(out=masks_sb, in_=maskb.ap())
            # 1 MB sub-chunks interleaved across the two big DMA queues in
            # exact consumption order, so the chunk needed next always has
            # the full bus: sync moves k0,v1,k2,v3 while gpsimd moves
            # v0,k1,v2,k3.
            k_sb = [
                singles.tile([P, 8, FT, P], MM_DT, name=f"k_sb{ci}")
                for ci in range(CI)
            ]
            v_sb = [
                singles.tile([P, 8, F], MM_DT, name=f"v_sb{ci}")
                for ci in range(CI)
            ]
            for ci in range(CI):
                k_eng = nc.sync if ci % 2 == 0 else nc.gpsimd
                v_eng = nc.gpsimd if ci % 2 == 0 else nc.sync
                if ci == 0:
                    # head-split: the first key/value tiles gate the first
                    # real matmuls right after warmup.  k0's first two
                    # tiles ride the otherwise-idle scalar queue so they
                    # don't queue behind q on sync.
                    nc.scalar.dma_start(
                        out=k_sb[ci][:, 0:1], in_=kb.ap()[ci, :, 0:1]
                    )
                    nc.scalar.dma_start(
                        out=k_sb[ci][:, 1:2], in_=kb.ap()[ci, :, 1:2]
                    )
                    for lo, hi in ((2, 4), (4, 8)):
                        k_eng.dma_start(
                            out=k_sb[ci][:, lo:hi], in_=kb.ap()[ci, :, lo:hi]
                        )
                    for lo, hi in ((0, 1), (1, 2), (2, 4), (4, 8)):
                        v_eng.dma_start(
                            out=v_sb[ci][:, lo:hi], in_=vb.ap()[ci, :, lo:hi]
                        )
                else:
                    k_eng.dma_start(out=k_sb[ci][:, :4], in_=kb.ap()[ci, :, :4])
                    k_eng.dma_start(out=k_sb[ci][:, 4:], in_=kb.ap()[ci, :, 4:])
                    v_eng.dma_start(out=v_sb[ci][:, :4], in_=vb.ap()[ci, :, :4])
                    v_eng.dma_start(out=v_sb[ci][:, 4:], in_=vb.ap()[ci, :, 4:])
            # gpsimd's DMA queue is serial, so these naturally wait behind
            # the v transfers — out of the critical first-window bandwidth
            projT_sb = singles.tile([P, FT, F], MM_DT)
            nc.gpsimd.dma_start(out=projT_sb, in_=projTb.ap())
            pbB_sb = singles.tile([P, F], F32)
            nc.gpsimd.dma_start(out=pbB_sb, in_=pbB.ap())
            ones_sb = singles.tile([P, 1], MM_DT)
            nc.vector.memset(ones_sb, 1.0)

            # pt arenas (bf16 attention weights), one per key chunk
            pt_ar = [
                singles.tile([P, 8, (CI - ci) * P], MM_DT, name=f"pt{ci}")
                for ci in range(CI)
            ]
            # zT arena: z^T[f, q] bf16, [P, ft, 512]
            zT_sb = singles.tile([P, FT, NL], MM_DT)
            rps = rpsum.tile([1, NL], F32)
            zA = [zpsum.tile([P, NL], F32, tag="zps", name=f"zA{h}") for h in range(4)]

            def scores(m):
                ci, u = divmod(m, 8)
                W = (CI - ci) * P
                ps = spsum.tile([P, W], F32, tag="sps")
                for ft in range(FT):
                    nc.tensor.matmul(
                        ps,
                        k_sb[ci][:, u, ft, :],
                        qT_sb[:, ft, ci * P : NL],
                        start=(ft == 0),
                        stop=(ft == FT - 1),
                    )
                pt = pt_ar[ci][:, u, :]
                nc.scalar.activation(
                    out=pt,
                    in_=ps,
                    func=mybir.ActivationFunctionType.Exp,
                    scale=SCALE,
                )
                # mask only the first 128 columns (slot ci — its diagonal chunk)
                nc.vector.tensor_mul(
                    out=pt_ar[ci][:, u, :P],
                    in0=pt_ar[ci][:, u, :P],
                    in1=masks_sb[:, u, :],
                )

            def attv(m, zt, fts):
                ci, u = divmod(m, 8)
                pt = pt_ar[ci][:, u, :]
                for i, ft in enumerate(fts):
                    nc.tensor.matmul(
                        zt[i][:, ci * P : NL],
                        v_sb[ci][:, u, ft * P : (ft + 1) * P],
                        pt,
                        start=(m == 0),
                        stop=(m == MT - 1),
                        skip_group_check=True,
                    )

            def rowsum(m):
                ci, u = divmod(m, 8)
                nc.tensor.matmul(
                    rps[:, ci * P : NL],
                    ones_sb,
                    pt_ar[ci][:, u, :],
                    start=(m == 0),
                    stop=(m == MT - 1),
                    skip_group_check=True,
                )

            # ---- pass A: scores + exp + mask + att@v (ft 0..3) + rowsums
            scores(0)
            for m in range(1, MT):
                scores(m)
                attv(m - 1, zA, range(4))
                rowsum(m - 1)
            attv(MT - 1, zA, range(4))
            rowsum(MT - 1)

            # row sums -> [q-part, slot] via DRAM round trip, then reciprocal
            rsum_row = singles.tile([1, NL], F32)
            nc.vector.tensor_copy(out=rsum_row, in_=rps)
            scratch = drampool.tile([1, NL], F32)
            nc.scalar.dma_start(out=scratch, in_=rsum_row)
            rsum_np = singles.tile([P, SLOTS], F32)
            nc.scalar.dma_start(
                out=rsum_np, in_=scratch[0].rearrange("(t p) -> p t", p=P)
            )
            recip_np = singles.tile([P, SLOTS], F32)
            nc.vector.reciprocal(out=recip_np, in_=rsum_np)

            # drain zT (ft 0..3)
            for h in range(4):
                if h % 2 == 0:
                    nc.vector.tensor_copy(out=zT_sb[:, h, :], in_=zA[h])
                else:
                    nc.scalar.activation(
                        out=zT_sb[:, h, :],
                        in_=zA[h],
                        func=mybir.ActivationFunctionType.Copy,
                    )

            # ---- pass B: att@v (ft 4..7)
            zB = [zpsum.tile([P, NL], F32, tag="zps", name=f"zB{h}") for h in range(4)]
            for m in range(MT):
                attv(m, zB, range(4, FT))
            for h in range(4):
                if h % 2 == 0:
                    nc.vector.tensor_copy(out=zT_sb[:, 4 + h, :], in_=zB[h])
                else:
                    nc.scalar.activation(
                        out=zT_sb[:, 4 + h, :],
                        in_=zB[h],
                        func=mybir.ActivationFunctionType.Copy,
                    )

            # ---- out[n, o] = (zT/rowsum) @ projT + pb
            for kk in range(SLOTS):
                for oc in range(2):
                    os_ = slice(oc * 512, (oc + 1) * 512)
                    ops = zpsum.tile([P, 512], F32, tag="zps", name="ops")
                    for ft in range(FT):
                        nc.tensor.matmul(
                            ops,
                            zT_sb[:, ft, kk * P : (kk + 1) * P],
                            projT_sb[:, ft, os_],
                            start=(ft == 0),
                            stop=(ft == FT - 1),
                        )
                    osb = opool.tile([P, 512], MM_DT, tag="osb")
                    nc.vector.scalar_tensor_tensor(
                        out=osb,
                        in0=ops,
                        scalar=recip_np[:, kk : kk + 1],
                        in1=pbB_sb[:, os_],
                        op0=mybir.AluOpType.mult,
                        op1=mybir.AluOpType.add,
                    )
                    nc.sync.dma_start(
                        out=out_o.ap()[kk * P : (kk + 1) * P, os_], in_=osb
                    )
    nc.finalize()
    return nc


def _get_programs():
    if "qkv" not in _CACHE:
        _CACHE["qkv"] = _build_qkv()
        _CACHE["attn"] = _build_attn()
    return _CACHE["qkv"], _CACHE["attn"]


def _c(a):
    return np.ascontiguousarray(a, dtype=np.float32)


def _b(a):
    return np.ascontiguousarray(np.asarray(a, dtype=np.float32).astype(ml_dtypes.bfloat16))


def kernel(x, wq_w, wq_b, wk_w, wk_b, wv_w, wv_b, proj_w, proj_b):
    x = np.asarray(x, dtype=np.float32)
    nc_qkv, nc_attn = _get_programs()

    # ---- launch A: QKV projection; core c owns query tiles {c, 8+c, 16+c, 24+c}
    # blocked weight layouts: [FT, P, DT, P] so every chunk DMA is linear
    wqb = _b(np.asarray(wq_w).T.reshape(DT, P, FT, P).transpose(2, 1, 0, 3))
    wkb = _b(np.asarray(wk_w).T.reshape(DT, P, FT, P).transpose(2, 1, 0, 3))
    wvb = _b(np.asarray(wv_w).T.reshape(DT, P, 2, 512).transpose(2, 1, 0, 3))
    bq_pb = _c(np.asarray(wq_b).reshape(FT, P).T)   # [P, FT]
    bk_pb = _c(np.asarray(wk_b).reshape(FT, P).T)
    bvB = _c(np.broadcast_to(np.asarray(wv_b), (P, F)))
    x_t = x.reshape(MT, P, D)                       # [tile, row, d]
    in_a = []
    for c in range(C):
        rows = x_t[c::C].reshape(NL, D)             # tiles c, 8+c, 16+c, 24+c
        xT_blk = _b(rows.T.reshape(DT, P, NL).transpose(1, 0, 2))
        in_a.append(
            {
                "xT": xT_blk,
                "wqb": wqb,
                "wkb": wkb,
                "wvb": wvb,
                "bq": bq_pb,
                "bk": bk_pb,
                "bvB": bvB,
            }
        )
    res_a = run_bass_kernel_spmd(nc_qkv, in_a, core_ids=list(range(C)))
    LAST_EXEC_NS[0] = res_a.exec_time_ns
    LAST_RESULTS[0] = res_a

    # reassemble full k/v in natural row order (tile index = 8*s + c)
    kT_all = np.stack(
        [np.asarray(res_a.results[c]["kT_o"]).reshape(F, SLOTS, P) for c in range(C)]
    )  # [c, F, s, P]
    kT_full = kT_all.transpose(1, 2, 0, 3).reshape(F, N)
    v_all = np.stack(
        [np.asarray(res_a.results[c]["v_o"]).reshape(SLOTS, P, F) for c in range(C)]
    )  # [c, s, P, F]
    v_full = v_all.transpose(1, 0, 2, 3).reshape(N, F)

    # ---- launch B: attention + projection (chunk-blocked, replicated k/v)
    # kb[ci, p, u, ft, j] = kT_full[ft*128 + p, (8*ci+u)*128 + j]
    kb = np.ascontiguousarray(
        kT_full.reshape(FT, P, CI, 8, P).transpose(2, 1, 3, 0, 4)
    )
    vb = np.ascontiguousarray(v_full.reshape(CI, 8, P, F).transpose(0, 2, 1, 3))
    projTb = _b(np.asarray(proj_w).T.reshape(FT, P, F).transpose(1, 0, 2))
    pbB = _c(np.broadcast_to(np.asarray(proj_b), (P, F)))
    tri = (np.arange(P)[:, None] <= np.arange(P)[None, :])  # key p <= query q
    in_b = []
    for c in range(C):
        qT_blk = np.ascontiguousarray(
            np.asarray(res_a.results[c]["qT_o"]).reshape(FT, P, NL).transpose(1, 0, 2)
        )
        maskb = np.zeros((P, 8, P), dtype=ml_dtypes.bfloat16)
        maskb[:, :c, :] = 1.0
        maskb[:, c, :] = tri.astype(ml_dtypes.bfloat16)
        in_b.append(
            {
                "qT": qT_blk,
                "kb": kb,
                "vb": vb,
                "maskb": maskb,
                "projTb": projTb,
                "pbB": pbB,
            }
        )
    res_b = run_bass_kernel_spmd(nc_attn, in_b, core_ids=list(range(C)))
    LAST_EXEC_NS[1] = res_b.exec_time_ns
    LAST_RESULTS[1] = res_b

    out_all = np.stack(
        [
            np.asarray(res_b.results[c]["out_o"]).astype(np.float32).reshape(SLOTS, P, F)
            for c in range(C)
        ]
    )  # [c, k, P, F]
    return out_all.transpose(1, 0, 2, 3).reshape(N, F)
